# revision 3
# baseline (speedup 1.0000x reference)
"""Trainium2 Bass kernel for nn_CSNeuralODE: 199-step Euler integration of a
controlled neural ODE, data-parallel over batch across 8 NeuronCores.

Layout: activations transposed ([features, batch]); per core batch 512 split
into 2 interleaved streams of 256 for cross-engine pipelining. Weights stay
resident in SBUF in fp32r (PE runs fp32r matmuls at full rate for N>=256).

Network transforms applied host-side (all exact algebra):
 - tanh(z) = 1 - 2*r with r = 1/(1+exp(2z)); the affine (1 - 2r) is folded
   into the next layer's weights/bias, so the device only computes
   r = recip(1 + exp(2z)) via ACT Exp + DVE add + DVE reciprocal_approx_fast.
 - softplus(z) = Ln(Exp(z) + 1) on ACT (exp and ln share one table set;
   native Softplus has no table on trn2, and Tanh's table would conflict).
 - L1 biases ride a constant ones-row appended to the state (K=65 matmul).
 - L2/L3 biases are added by one K=2 matmul against a constant selector.
 - g-branch: u(t)*g folds sin(t*freqs) into per-step G2 weights; all
   per-step constants (bf3 + u*(colsum(Wg1)+bg1)) ride row 52 of the G2
   stationary operand against a constant-1.0 row of r (generated by a
   bias=-50 padding column in L1).
"""

import os
import numpy as np

D = 64
H = 256
HG = 52
B = 4096
T = 200
NCORES = 8
BS = B // NCORES      # batch per core = 512
NSTREAM = 2
NS = BS // NSTREAM    # batch per stream = 256

_CACHE = {}


def _build(n_steps, dt):
    import concourse.bass as bass
    import concourse.bacc as bacc
    import concourse.mybir as mybir
    import concourse.tile as tile
    from concourse.dve_ops import RECIPROCAL_APPROX_FAST, RECIP_APPROX_FAST_CONSTS

    F32 = mybir.dt.float32
    F32R = mybir.dt.float32r
    AF = mybir.ActivationFunctionType
    RC = RECIP_APPROX_FAST_CONSTS

    nc = bacc.Bacc("TRN2", target_bir_lowering=False, debug=False,
                   num_devices=NCORES)

    # ---- DRAM I/O ----
    d_y0r = nc.dram_tensor("y0r", [65, BS], F32R, kind="ExternalInput")
    d_y0f = nc.dram_tensor("y0f", [64, BS], F32, kind="ExternalInput")
    d_wl1 = nc.dram_tensor("wl1", [65, 384], F32R, kind="ExternalInput")
    d_wl2 = nc.dram_tensor("wl2", [128, 512], F32R, kind="ExternalInput")
    d_wl3 = nc.dram_tensor("wl3", [128, 512], F32R, kind="ExternalInput")
    d_wl4 = nc.dram_tensor("wl4", [128, 128], F32R, kind="ExternalInput")
    d_bw = nc.dram_tensor("bw", [2, 256], F32R, kind="ExternalInput")
    d_sel = nc.dram_tensor("sel", [2, 512], F32R, kind="ExternalInput")
    d_g2 = nc.dram_tensor("g2", [53, 64 * n_steps], F32R, kind="ExternalInput")
    d_out = nc.dram_tensor("yT", [64, BS], F32, kind="ExternalOutput")

    with tile.TileContext(nc) as tc:
        with (
            tc.tile_pool(name="w", bufs=1) as wp,
            tc.tile_pool(name="pl1", bufs=2, space="PSUM") as pl1,
            tc.tile_pool(name="pmid", bufs=2, space="PSUM") as pmid,
            tc.tile_pool(name="pf", bufs=2, space="PSUM") as pf,
            tc.tile_pool(name="pe1", bufs=2) as pe1,
            tc.tile_pool(name="pd", bufs=2) as pd,
            tc.tile_pool(name="pr", bufs=2) as pr,
            tc.tile_pool(name="pe2", bufs=2) as pe2,
            tc.tile_pool(name="psp", bufs=2) as psp,
        ):
            t_wl1 = wp.tile([65, 384], F32R)
            nc.sync.dma_start(t_wl1[:], d_wl1[:, :])
            t_wl2 = wp.tile([128, 512], F32R)
            nc.sync.dma_start(t_wl2[:], d_wl2[:, :])
            t_wl3 = wp.tile([128, 512], F32R)
            nc.sync.dma_start(t_wl3[:], d_wl3[:, :])
            t_wl4 = wp.tile([128, 128], F32R)
            nc.sync.dma_start(t_wl4[:], d_wl4[:, :])
            t_bw = wp.tile([2, 256], F32R)
            nc.sync.dma_start(t_bw[:], d_bw[:, :])
            t_sel = wp.tile([2, 512], F32R)
            nc.sync.dma_start(t_sel[:], d_sel[:, :])
            t_g2 = wp.tile([53, 64 * n_steps], F32R)
            nc.sync.dma_start(t_g2[:], d_g2[:, :])

            t_yr = []
            t_ys = []  # [stream][parity]
            for s in range(NSTREAM):
                yr = wp.tile([65, NS], F32R, tag=f"yr{s}")
                nc.sync.dma_start(yr[:], d_y0r[:, s * NS:(s + 1) * NS])
                t_yr.append(yr)
                ya = wp.tile([64, NS], F32, tag=f"ya{s}")
                nc.sync.dma_start(ya[:], d_y0f[:, s * NS:(s + 1) * NS])
                yb = wp.tile([64, NS], F32, tag=f"yb{s}")
                t_ys.append([ya, yb])

            for n in range(n_steps):
                for s in range(NSTREAM):
                    y_cur = t_ys[s][n % 2]
                    y_nxt = t_ys[s][(n + 1) % 2]
                    yr = t_yr[s]

                    # ---- L1 (+G1): z1 = W0aug.T @ [y;1] ----
                    ps1 = pl1.tile([128, 3 * NS], F32)
                    for c in range(3):
                        nc.tensor.matmul(
                            ps1[:, c * NS:(c + 1) * NS],
                            t_wl1[:, c * 128:(c + 1) * 128],
                            yr[:],
                            start=True, stop=True)
                    # e1 = exp(2*z1)
                    e1 = pe1.tile([128, 3 * NS], F32)
                    nc.scalar.activation(e1[:], ps1[:], AF.Exp, scale=2.0)
                    # r = 1/(1+e1), per chunk for finer pipelining
                    rts = []
                    for c in range(3):
                        dtile = pd.tile([128, NS], F32, tag=f"d{c}")
                        nc.vector.tensor_scalar_add(
                            dtile[:], e1[:, c * NS:(c + 1) * NS], 1.0)
                        rtile = pr.tile([128, NS], F32R, tag=f"r{c}")
                        nc.vector._custom_dve(
                            RECIPROCAL_APPROX_FAST, out=rtile[:], in0=dtile[:],
                            s0=RC["s0"], s1=RC["s1"], imm2=RC["imm2"])
                        rts.append(rtile)

                    # ---- L2: z2 = A1.T @ r + c1 ----
                    ps2 = pmid.tile([128, 2 * NS], F32, tag="psmid")
                    for m in range(2):
                        nc.tensor.matmul(
                            ps2[:, m * NS:(m + 1) * NS],
                            t_wl2[:, 128 * (0 + m):128 * (1 + m)],
                            rts[0][:], start=True, stop=False,
                            skip_group_check=True)
                        nc.tensor.matmul(
                            ps2[:, m * NS:(m + 1) * NS],
                            t_wl2[:, 128 * (2 + m):128 * (3 + m)],
                            rts[1][:], start=False, stop=False,
                            skip_group_check=True)
                    nc.tensor.matmul(
                        ps2[:], t_bw[:, 0:128], t_sel[:],
                        start=False, stop=True, skip_group_check=True)
                    e2 = pe2.tile([128, 2 * NS], F32, tag="e2")
                    nc.scalar.activation(e2[:], ps2[:], AF.Exp)
                    sp2 = psp.tile([128, 2 * NS], F32R, tag="sp2")
                    nc.scalar.activation(sp2[:], e2[:], AF.Ln, bias=1.0)

                    # ---- L3: z3 = W2.T @ sp2 + c2 ----
                    ps3 = pmid.tile([128, 2 * NS], F32, tag="psmid")
                    for m in range(2):
                        nc.tensor.matmul(
                            ps3[:, m * NS:(m + 1) * NS],
                            t_wl3[:, 128 * (0 + m):128 * (1 + m)],
                            sp2[:, 0:NS], start=True, stop=False,
                            skip_group_check=True)
                        nc.tensor.matmul(
                            ps3[:, m * NS:(m + 1) * NS],
                            t_wl3[:, 128 * (2 + m):128 * (3 + m)],
                            sp2[:, NS:2 * NS], start=False, stop=False,
                            skip_group_check=True)
                    nc.tensor.matmul(
                        ps3[:], t_bw[:, 128:256], t_sel[:],
                        start=False, stop=True, skip_group_check=True)
                    e3 = pe2.tile([128, 2 * NS], F32, tag="e3")
                    nc.scalar.activation(e3[:], ps3[:], AF.Exp)
                    sp3 = psp.tile([128, 2 * NS], F32R, tag="sp3")
                    nc.scalar.activation(sp3[:], e3[:], AF.Ln, bias=1.0)

                    # ---- L4 + G2: f_total = Wf3.T @ sp3 + G2(n).T @ rg ----
                    psf = pf.tile([64, NS], F32)
                    nc.tensor.matmul(psf[:], t_wl4[:, 0:64], sp3[:, 0:NS],
                                     start=True, stop=False,
                                     skip_group_check=True)
                    nc.tensor.matmul(psf[:], t_wl4[:, 64:128], sp3[:, NS:2 * NS],
                                     start=False, stop=False,
                                     skip_group_check=True)
                    nc.tensor.matmul(psf[:], t_g2[:, n * 64:(n + 1) * 64],
                                     rts[2][0:53, :],
                                     start=False, stop=True,
                                     skip_group_check=True)

                    # ---- y update ----
                    nc.vector.affine_then_add(out=y_nxt[:], in0=psf[:],
                                              in1=y_cur[:], scale=float(dt),
                                              bias=0.0)
                    nc.vector.tensor_copy(yr[0:64, :], y_nxt[:])

            for s in range(NSTREAM):
                nc.sync.dma_start(d_out[:, s * NS:(s + 1) * NS],
                                  t_ys[s][n_steps % 2][:])

    nc.compile()
    return nc


def _prepare_host(inputs, n_steps):
    t = np.asarray(inputs["t"], np.float32)
    dt = float(np.float32(t[1] - t[0]))
    freqs = np.arange(1, D + 1, dtype=np.float32)

    Wf0 = np.asarray(inputs["Wf0"], np.float32)
    bf0 = np.asarray(inputs["bf0"], np.float32)
    Wf1 = np.asarray(inputs["Wf1"], np.float32)
    bf1 = np.asarray(inputs["bf1"], np.float32)
    Wf2 = np.asarray(inputs["Wf2"], np.float32)
    bf2 = np.asarray(inputs["bf2"], np.float32)
    Wf3 = np.asarray(inputs["Wf3"], np.float32)
    bf3 = np.asarray(inputs["bf3"], np.float32)
    Wg0 = np.asarray(inputs["Wg0"], np.float32)
    bg0 = np.asarray(inputs["bg0"], np.float32)
    Wg1 = np.asarray(inputs["Wg1"], np.float32)
    bg1 = np.asarray(inputs["bg1"], np.float32)

    # L1 augmented weights [65, 384]; tanh layers receive 2x scale at the ACT.
    wl1 = np.zeros((65, 384), np.float32)
    wl1[:64, 0:256] = Wf0
    wl1[64, 0:256] = bf0
    wl1[:64, 256:308] = Wg0
    wl1[64, 256:308] = bg0
    wl1[64, 308] = -50.0        # r-row generator: r[52] == 1.0 exactly

    # L2 on r: z2 = (-2 Wf1).T @ r + (bf1 + colsum(Wf1))
    A1 = (-2.0 * Wf1).astype(np.float32)
    c1 = (bf1 + Wf1.sum(axis=0)).astype(np.float32)
    wl2 = np.zeros((128, 512), np.float32)
    for k in range(2):
        for m in range(2):
            wl2[:, 128 * (2 * k + m):128 * (2 * k + m + 1)] = \
                A1[128 * k:128 * (k + 1), 128 * m:128 * (m + 1)]

    wl3 = np.zeros((128, 512), np.float32)
    for k in range(2):
        for m in range(2):
            wl3[:, 128 * (2 * k + m):128 * (2 * k + m + 1)] = \
                Wf2[128 * k:128 * (k + 1), 128 * m:128 * (m + 1)]
    c2 = bf2.astype(np.float32)

    bw = np.zeros((2, 256), np.float32)
    bw[0, 0:128] = c1[0:128]
    bw[1, 0:128] = c1[128:256]
    bw[0, 128:256] = c2[0:128]
    bw[1, 128:256] = c2[128:256]

    sel = np.zeros((2, 512), np.float32)
    sel[0, 0:256] = 1.0
    sel[1, 256:512] = 1.0

    wl4 = np.zeros((128, 128), np.float32)
    wl4[:, 0:64] = Wf3[0:128, :]
    wl4[:, 64:128] = Wf3[128:256, :]

    # G2 per-step stationary [53, 64] blocks: rows 0-51 = -2*Wg1*u_n,
    # row 52 = bf3 + u_n*(colsum(Wg1)+bg1)
    colg = Wg1.sum(axis=0).astype(np.float32)
    g2 = np.zeros((53, 64 * n_steps), np.float32)
    for n in range(n_steps):
        u = np.sin(t[n] * freqs).astype(np.float32)
        g2[0:52, 64 * n:64 * (n + 1)] = (-2.0 * Wg1) * u[None, :]
        g2[52, 64 * n:64 * (n + 1)] = bf3 + u * (colg + bg1)

    shared = {"wl1": wl1, "wl2": wl2, "wl3": wl3, "wl4": wl4,
              "bw": bw, "sel": sel, "g2": g2}
    return shared, dt


def kernel(**inputs):
    from concourse.bass_utils import run_bass_kernel_spmd

    n_steps = len(np.asarray(inputs["t"])) - 1
    shared, dt = _prepare_host(inputs, n_steps)

    key = (n_steps, dt)
    if key not in _CACHE:
        _CACHE[key] = _build(n_steps, dt)
    nc = _CACHE[key]

    y0 = np.asarray(inputs["y0"], np.float32).reshape(B, D)
    in_maps = []
    for c in range(NCORES):
        shard = y0[c * BS:(c + 1) * BS, :]            # [BS, 64]
        ytr = np.ascontiguousarray(shard.T)           # [64, BS]
        y0r = np.concatenate([ytr, np.ones((1, BS), np.float32)], axis=0)
        m = dict(shared)
        m["y0r"] = y0r
        m["y0f"] = ytr
        in_maps.append(m)

    res = run_bass_kernel_spmd(nc, in_maps, core_ids=list(range(NCORES)))
    out = np.empty((B, D), np.float32)
    for c in range(NCORES):
        out[c * BS:(c + 1) * BS, :] = res.results[c]["yT"].T
    return out.reshape(B, 1, D)
